# revision 26
# baseline (speedup 1.0000x reference)
"""Masked mean-pool (NonZeroAvgPool) Trainium2 Bass kernel.

out[b, d] = sum_s (tokens[b,s] != 0) * x[b,s,d] / sum_s (tokens[b,s] != 0)

Full shapes: x [16, 4096, 512] f32, tokens [16, 4096] i32 -> out [16, 512] f32.
Sharding: pure data parallel over batch; 2 batches per core on 8 cores.

Per-core program (shapes [2, 4096, 512] / [2, 4096] -> [2, 512]):
  - sequence rows are indexed s = p*32 + c  (p: SBUF partition, c: chunk)
    so every DMA is contiguous per partition.
  - valid[p, c]  = (tokens != 0) as f32 via DVE not_equal
  - counts[1, 2] = ones[128,1].T @ rowsum(valid)        (PE, one matmul)
  - num[1, D]    = sum_c valid[:, c].T @ x_tile[:, c, :] (PE, PSUM accumulate)
  - out row      = num * (1/count) on ACT (reads PSUM) and per-batch store

Two implementations are kept: the default RAW path (K_RAW=1, no
TileContext, hand-placed semaphores) and the Tile path (K_RAW=0) it was
derived from. Raw is ~1.7us faster: the ACT engine's first main-block
instruction is the first x-DMA trigger (Tile's entry barrier delayed it
~0.8us) and there is no Tile exit sem ladder (~1.2us). SBUF holds both
batches' x (64 chunks x 2KB = 128KB/partition), so no buffer reuse and
only ~8 cross-engine semaphore edges are needed (see _raw_body docstring).

Measured structure (ntff traces, exec window = gauge first..last_useful),
raw path, official 53329ns:
  exec = head (~2.3us: gpsimd init sem-clear fence + trigger + HBM latency)
       + x stream (16.78MB at 400-417 B/ns solo, ~345-351 B/ns when the
         paired NeuronCore streams concurrently -- pure launch-skew luck)
       + overhang (~2us: last DMA's completion receipt straggles 1.2-1.8us
         behind its last byte -- physics, the final matmul must wait it)
       + out chain (~2.6us: ACT divide 0.8 + trigger + flight + receipt)
       + FIXED postamble (~6.5us: ~51 per-sem zero-writes per engine at
         ~115ns each behind the final all-engine barrier, + $S[2] ladder).
  A trivial 3-instruction Tile kernel measures 14.5us total overhead;
  the raw path's non-stream overhead is ~14.1us with ~4.6us of it the
  unavoidable receipt latencies + sweep start gating on out completion.

Key corrections to earlier findings (verified on HW this session):
  - DMAHW sem-lane reuse is NOT fatal: 12+ DMAs per core (lanes reused
    round-robin) run fine. The old "max 8 DMAs" rule was wrong.
  - The tail taper 18,8,4,1,1 exists so almost no PE work remains after
    the last byte lands: each late group's matmuls (427ns/mm at the
    HAM-cold 1.2GHz clock) hide under the next group's DMA (766ns/chunk
    contended, ~620 solo). Post-stream matmul overhang ~0.9us.
  - gpsimd custom-ucode paths (dma_gather / sparse_gather /
    indirect_dma_start) all crash NRT_EXEC_UNIT_UNRECOVERABLE here:
    running_on_bedrock()=True and the HIPI/library ucode is excluded
    from this image, so only base-firmware ops (plain dma_start, iota)
    work. A valid-row gather (~2/3 of rows -> 11.2MB traffic) is the
    obvious next win if ucode ever appears.
"""

from contextlib import ExitStack

import numpy as np

import concourse.bacc as bacc
import concourse.bass as bass
import concourse.tile as tile
from concourse import mybir
from concourse.bass_utils import run_bass_kernel_spmd

B, S, D = 16, 4096, 512
NCORES = 8
BPC = B // NCORES  # batches per core = 2
P = 128            # SBUF partitions
CPB = S // P       # chunks per batch = 32
import os

# Tapered x-DMA groups per batch (chunks of 256KB each). Lane reuse beyond
# the 8 DMAHW sem lanes is safe (verified on HW); the taper minimizes the
# PE-matmul chain left after the final byte lands (see module docstring).
GROUPS = [int(g) for g in os.environ.get("K_GROUPS", "18,8,4,1,1").split(",")]
assert sum(GROUPS) == CPB
# x-loads issue on the ACT HWDGE ring: reproducibly ~0.7us faster than sync
# (58.8/58.8 vs 59.4/59.6 us) — tok/out stay on SP, splitting sequencer work.
X_ENGINE = os.environ.get("K_XENG", "act")  # sync | act | gpsimd
# Tiles are tagged by group size; BPC bufs per tag gives every x DMA a private
# slot (no reuse -> no extra WAW waits). Total SBUF: 2*(16+15+1)*2KB = 128KB/p.
XBUFS = BPC
# PE HAM warming: "pre,per-group" dummy fp32 N=1 matmuls on `ones`. PE idles
# ~3.4us between DMA-paced groups, exactly the HAM re-gate window, so real
# matmuls run at the cold 1.2 GHz clock; dummies keep the PE busy -> 2.4 GHz.
WARM0, WARMG = (int(v) for v in os.environ.get("K_WARM", "0,0").split(","))

_NC = None


RAW = os.environ.get("K_RAW", "1") == "1"


def _build_nc():
    # Bacc (not plain Bass): its compile() runs generate_event_semaphores,
    # which splits multi-wait instructions onto InstEventSemaphore — TRN2
    # instructions can carry at most one sem wait.
    nc = bacc.Bacc(trn_type="TRN2")
    x = nc.dram_tensor("x", [BPC, S, D], mybir.dt.float32, kind="ExternalInput")
    tokens = nc.dram_tensor("tokens", [BPC, S], mybir.dt.int32, kind="ExternalInput")
    out = nc.dram_tensor("out", [BPC, D], mybir.dt.float32, kind="ExternalOutput")

    # s = p*CPB + c : per-partition contiguous rows
    xa = x[:].rearrange("b (p c) d -> b p c d", p=P)   # [BPC, 128, 32, 512]
    ta = tokens[:].rearrange("b (p c) -> p b c", p=P)  # [128, BPC, 32]
    oa = out[:].rearrange("b d -> (b d)")              # [BPC*512]

    if RAW:
        _raw_body(nc, xa, ta, oa)
    else:
        with TileKernel(nc) as tk:
            tk.body(xa, ta, oa)
    nc.compile()
    return nc


def _raw_body(nc, xa, ta, oa):
    """Hand-scheduled variant: no TileContext, manual semaphores.

    Why: Tile's entry barrier + event-sem config delays the first x-DMA
    trigger by ~0.8-1.4us after the measured window opens, and its exit adds
    a sem ladder. Raw emission lets the ACT engine's FIRST instruction be
    the first x-DMA trigger. SBUF is big enough to hold both batches'
    x (64 chunks x 2KB = 128KB/partition), so there is no buffer reuse and
    the only cross-engine sync needed is:

      SP:  tok DMA -> tsem          ACT: 10 x-DMA triggers -> xsem (16/DMA)
      DVE: [tsem] valid, rowsum -> vsem; [csem] recips -> rsem
      PE:  [vsem] cnt matmul -> csem; per group g: [xsem >= 16*g] matmuls;
           per-batch last matmul -> nsem
      ACT: [rsem, nsem>=b+1] orow_b = num_b * recip_b (reads PSUM) -> osem
      SP:  [osem >= b+1] store orow_b

    Cumulative xsem waits are sound on one HWDGE ring: each of the 16 SDMA
    engines emits 1 inc per DMA and processes its queue in ring order, so
    xsem >= 16*k implies every engine finished its share of the first k
    DMAs, i.e. DMAs 1..k fully landed.
    """
    with ExitStack() as es:
        sb = lambda name, shape, dt: es.enter_context(nc.sbuf_tensor(name, shape, dt))
        ps = lambda name, shape, dt: es.enter_context(nc.psum_tensor(name, shape, dt))
        sem = lambda name: es.enter_context(nc.semaphore(name))

        xsb = sb("xsb", [P, BPC * CPB, D], mybir.dt.float32r)  # both batches
        tok = sb("tok", [P, BPC, CPB], mybir.dt.int32)
        valid = sb("valid", [P, BPC, CPB], mybir.dt.float32r)
        rowsum = sb("rowsum", [P, BPC], mybir.dt.float32)
        recips = sb("recips", [1, BPC], mybir.dt.float32)
        orow = sb("orow", [1, BPC * D], mybir.dt.float32)
        ones = sb("ones", [P, 1], mybir.dt.float32)
        cnt = ps("cnt", [1, BPC], mybir.dt.float32)
        nums = [ps(f"num{b}", [1, D], mybir.dt.float32) for b in range(BPC)]

        nx = BPC * len(GROUPS)
        xsems = [sem(f"xsem{i}") for i in range(nx)]
        tsem = sem("tsem")
        vsem = sem("vsem")
        csem = sem("csem")
        rsem = sem("rsem")
        nsem = sem("nsem")
        osem = sem("osem")

        # --- ACT: the x stream, queued up-front; first trigger is ACT's
        # first main-block instruction -------------------------------------
        di = 0
        for b in range(BPC):
            c0 = 0
            for grp in GROUPS:
                nc.scalar.dma_start(
                    out=xsb[:, b * CPB + c0:b * CPB + c0 + grp, :],
                    in_=xa[b, :, c0:c0 + grp, :].bitcast(mybir.dt.float32r),
                ).then_inc(xsems[di], 16)
                di += 1
                c0 += grp

        # --- SP: token load ------------------------------------------------
        nc.sync.dma_start(out=tok[:], in_=ta).then_inc(tsem, 16)

        # --- DVE: mask, row sums, reciprocals ------------------------------
        # dsem: the race model doesn't credit same-engine program order, so
        # DVE's own write->read chain gets an explicit handshake.
        dsem = sem("dsem")
        nc.vector.memset(ones[:], 1.0).then_inc(dsem, 1)
        nc.vector.wait_ge(tsem, 16)
        nc.vector.tensor_scalar(
            out=valid[:], in0=tok[:], scalar1=0, scalar2=None,
            op0=mybir.AluOpType.not_equal,
        ).then_inc(dsem, 1)
        nc.vector.wait_ge(dsem, 2)
        nc.vector.reduce_sum(
            out=rowsum[:], in_=valid[:].bitcast(mybir.dt.float32),
            axis=mybir.AxisListType.X,
        ).then_inc(vsem, 1)
        nc.vector.wait_ge(csem, 1)
        nc.vector.reciprocal(recips[:], cnt[:]).then_inc(rsem, 1)

        # --- PE: counts, then the masked-sum groups ------------------------
        nc.tensor.wait_ge(vsem, 1)
        nc.tensor.matmul(cnt[:], ones[:], rowsum[:], start=True, stop=True
                         ).then_inc(csem, 1)
        dma_idx = 0
        for b in range(BPC):
            c0 = 0
            for grp in GROUPS:
                nc.tensor.wait_ge(xsems[dma_idx], 16)
                dma_idx += 1
                for k in range(grp):
                    c = c0 + k
                    mm = nc.tensor.matmul(
                        nums[b][:], valid[:, b, c:c + 1],
                        xsb[:, b * CPB + c, :],
                        start=(c == 0), stop=(c == CPB - 1),
                    )
                    if c == CPB - 1:
                        mm.then_inc(nsem, 1)
                c0 += grp

        # --- ACT: divide straight out of PSUM ------------------------------
        # (A split-divide across ACT+DVE and dropping the final fsem waits
        # were both tried; HW runs failed with redacted INTERNAL errors, so
        # this block stays exactly as the proven 53329ns version.)
        nc.scalar.wait_ge(rsem, 1)
        for b in range(BPC):
            nc.scalar.wait_ge(nsem, b + 1)
            nc.scalar.activation(
                orow[:, b * D:(b + 1) * D], nums[b][:],
                mybir.ActivationFunctionType.Copy, scale=recips[:, b:b + 1],
            ).then_inc(osem, 1)

        # --- SP: per-batch stores ------------------------------------------
        # The final fsem waits are REQUIRED: ending the program with these
        # DMAs in flight crashes the runtime at ring teardown (tested).
        fsems = [sem(f"fsem{b}") for b in range(BPC)]
        for b in range(BPC):
            nc.sync.wait_ge(osem, b + 1)
            nc.sync.dma_start(
                out=oa[b * D:(b + 1) * D], in_=orow[:, b * D:(b + 1) * D]
            ).then_inc(fsems[b], 16)
        for b in range(BPC):
            nc.sync.wait_ge(fsems[b], 16)


class TileKernel:
    def __init__(self, nc):
        self.nc = nc
        self.ctx = ExitStack()
        self.tc = None

    def __enter__(self):
        self.tc = self.ctx.enter_context(tile.TileContext(self.nc))
        return self

    def __exit__(self, *exc):
        return self.ctx.__exit__(*exc)

    def body(self, xa, ta, oa, xt0=None, xsem=None):
        nc = self.nc
        tc = self.tc
        ctx = self.ctx

        xpool = ctx.enter_context(tc.tile_pool(name="xpool", bufs=XBUFS))
        vpool = ctx.enter_context(tc.tile_pool(name="vpool", bufs=2))
        spool = ctx.enter_context(tc.tile_pool(name="spool", bufs=2))
        singles = ctx.enter_context(tc.tile_pool(name="singles", bufs=1))
        psum = ctx.enter_context(tc.tile_pool(name="psum", bufs=2, space="PSUM"))

        ones = singles.tile([P, 1], mybir.dt.float32)
        nc.vector.memset(ones, 1.0)
        xeng = {"sync": nc.sync, "act": nc.scalar, "gpsimd": nc.gpsimd}[X_ENGINE]

        warm = (
            psum.tile([1, 1], mybir.dt.float32, tag="warm", name="warm")
            if WARM0 or WARMG else None
        )

        def warm_pe(n):
            for _ in range(n):
                nc.tensor.matmul(warm, ones, ones, start=True, stop=True)

        warm_pe(WARM0)  # cover the initial DMA fill phase

        # --- mask + counts for both batches (one tok DMA) --------------------
        tok = vpool.tile([P, BPC, CPB], mybir.dt.int32)
        nc.sync.dma_start(out=tok, in_=ta)
        # valid is declared float32r so the fp32r matmul's verifier sees a
        # rounded producer; its values (0.0/1.0) are exact in any precision.
        valid = vpool.tile([P, BPC, CPB], mybir.dt.float32r)
        nc.vector.tensor_scalar(
            out=valid, in0=tok, scalar1=0, scalar2=None,
            op0=mybir.AluOpType.not_equal,
        )
        rowsum = spool.tile([P, BPC], mybir.dt.float32)
        nc.vector.reduce_sum(
            out=rowsum, in_=valid.bitcast(mybir.dt.float32),
            axis=mybir.AxisListType.X,
        )

        # both batches' counts in one matmul, reciprocals in one DVE op
        cnt = psum.tile([1, BPC], mybir.dt.float32)
        nc.tensor.matmul(cnt, ones, rowsum, start=True, stop=True)
        recips = spool.tile([1, BPC], mybir.dt.float32)
        nc.vector.reciprocal(recips, cnt)

        for b in range(BPC):
            recip = recips[:, b:b + 1]

            # --- masked sum ---------------------------------------------------
            num = psum.tile([1, D], mybir.dt.float32)
            c0 = 0
            for gi, grp in enumerate(GROUPS):
                if b == 0 and gi == 0 and xt0 is not None:
                    # first group was DMA'd by the hoisted pre-Tile transfer
                    nc.tensor.wait_ge(xsem, 16)
                    xt = xt0
                else:
                    # float32r: single-pass fp32 matmul (4x faster than fp32's
                    # two half-rate passes). Same 4-byte layout as fp32 so the
                    # DMA is a pure bit copy; the PE truncates low mantissa
                    # bits, mask weights are exact 0/1, PSUM accums in fp32.
                    xt = xpool.tile([P, grp, D], mybir.dt.float32r, tag=f"xt{grp}")
                    xeng.dma_start(out=xt, in_=xa[b, :, c0:c0 + grp, :].bitcast(mybir.dt.float32r))
                for k in range(grp):
                    c = c0 + k
                    nc.tensor.matmul(
                        num, valid[:, b, c:c + 1], xt[:, k, :],
                        start=(c == 0), stop=(c == CPB - 1),
                    )
                c0 += grp
                # keep PE busy through the wait for the next group's DMA,
                # except on the home stretch where dummies would delay the tail
                if not (b == BPC - 1 and c0 > CPB - GROUPS[-1] - GROUPS[-2]):
                    warm_pe(WARMG)

            # --- divide + store this batch's row as soon as it's ready --------
            # ACT reads PSUM directly: out = num * recip in one op, then store.
            orow = spool.tile([1, D], mybir.dt.float32, tag=f"orow{b}")
            nc.scalar.activation(
                orow, num, mybir.ActivationFunctionType.Copy, scale=recip
            )
            nc.sync.dma_start(out=oa[b * D:(b + 1) * D], in_=orow)


def _get_nc():
    global _NC
    if _NC is None:
        _NC = _build_nc()
    return _NC


def _shard(x, tokens):
    x = np.ascontiguousarray(np.asarray(x, dtype=np.float32))
    tokens = np.ascontiguousarray(np.asarray(tokens, dtype=np.int32))
    return [
        {
            "x": x[c * BPC:(c + 1) * BPC],
            "tokens": tokens[c * BPC:(c + 1) * BPC],
        }
        for c in range(NCORES)
    ]


def kernel(x, tokens):
    res = run_bass_kernel_spmd(_get_nc(), _shard(x, tokens), core_ids=list(range(NCORES)))
    return np.concatenate([r["out"] for r in res.results], axis=0)


def _install_ntff_shim():
    """The agent image's antenv lacks axon_hooks, so bass_utils' trace path
    can't find the NTFF hook. Recreate the tiny get/set module and register
    trn_boot's ctypes-based hook against the injected libaxon_pjrt.so."""
    import sys
    import types

    if "antenv.axon_hooks" in sys.modules:
        return
    mod = types.ModuleType("antenv.axon_hooks")
    state = {"hook": None}
    mod.set_axon_ntff_profile_hook = lambda h: state.__setitem__("hook", h)
    mod.get_axon_ntff_profile_hook = lambda: state["hook"]
    sys.modules["antenv.axon_hooks"] = mod
    try:
        from trn_agent_boot.trn_boot import _ntff_profile_via_ctypes

        mod.set_axon_ntff_profile_hook(
            _ntff_profile_via_ctypes("/opt/axon/libaxon_pjrt.so")
        )
    except Exception:
        pass


def kernel_profiled(x, tokens):
    """Same as kernel() but with NTFF tracing; returns (out, BassKernelResults)."""
    _install_ntff_shim()
    res = run_bass_kernel_spmd(
        _get_nc(), _shard(x, tokens), core_ids=list(range(NCORES)), trace=True
    )
    out = np.concatenate([r["out"] for r in res.results], axis=0)
    return out, res



# revision 27
# speedup vs baseline: 1.2267x; 1.2267x over previous
"""Masked mean-pool (NonZeroAvgPool) Trainium2 Bass kernel.

out[b, d] = sum_s (tokens[b,s] != 0) * x[b,s,d] / sum_s (tokens[b,s] != 0)

Full shapes: x [16, 4096, 512] f32, tokens [16, 4096] i32 -> out [16, 512] f32.
Sharding: pure data parallel over batch; 2 batches per core on 8 cores.

Per-core program (shapes [2, 4096, 512] / [2, 4096] -> [2, 512]):
  - sequence rows are indexed s = p*32 + c  (p: SBUF partition, c: chunk)
    so every DMA is contiguous per partition.
  - valid[p, c]  = (tokens != 0) as f32 via DVE not_equal
  - counts[1, 2] = ones[128,1].T @ rowsum(valid)        (PE, one matmul)
  - num[1, D]    = sum_c valid[:, c].T @ x_tile[:, c, :] (PE, PSUM accumulate)
  - out row      = num * (1/count) on ACT (reads PSUM) and per-batch store

Two implementations are kept: the default RAW path (K_RAW=1, no
TileContext, hand-placed semaphores) and the Tile path (K_RAW=0) it was
derived from. Raw is ~1.7us faster: the ACT engine's first main-block
instruction is the first x-DMA trigger (Tile's entry barrier delayed it
~0.8us) and there is no Tile exit sem ladder (~1.2us). SBUF holds both
batches' x (64 chunks x 2KB = 128KB/partition), so no buffer reuse and
only ~8 cross-engine semaphore edges are needed (see _raw_body docstring).

Measured structure (ntff traces, exec window = gauge first..last_useful),
raw path, official 53329ns:
  exec = head (~2.3us: gpsimd init sem-clear fence + trigger + HBM latency)
       + x stream (16.78MB at 400-417 B/ns solo, ~345-351 B/ns when the
         paired NeuronCore streams concurrently -- pure launch-skew luck)
       + overhang (~2us: last DMA's completion receipt straggles 1.2-1.8us
         behind its last byte -- physics, the final matmul must wait it)
       + out chain (~2.6us: ACT divide 0.8 + trigger + flight + receipt)
       + FIXED postamble (~6.5us: ~51 per-sem zero-writes per engine at
         ~115ns each behind the final all-engine barrier, + $S[2] ladder).
  A trivial 3-instruction Tile kernel measures 14.5us total overhead;
  the raw path's non-stream overhead is ~14.1us with ~4.6us of it the
  unavoidable receipt latencies + sweep start gating on out completion.

Key corrections to earlier findings (verified on HW this session):
  - DMAHW sem-lane reuse is NOT fatal: 12+ DMAs per core (lanes reused
    round-robin) run fine. The old "max 8 DMAs" rule was wrong.
  - The tail taper 18,8,4,1,1 exists so almost no PE work remains after
    the last byte lands: each late group's matmuls (427ns/mm at the
    HAM-cold 1.2GHz clock) hide under the next group's DMA (766ns/chunk
    contended, ~620 solo). Post-stream matmul overhang ~0.9us.
  - gpsimd custom-ucode paths (dma_gather / sparse_gather /
    indirect_dma_start) all crash NRT_EXEC_UNIT_UNRECOVERABLE here:
    running_on_bedrock()=True and the HIPI/library ucode is excluded
    from this image, so only base-firmware ops (plain dma_start, iota)
    work. A valid-row gather (~2/3 of rows -> 11.2MB traffic) is the
    obvious next win if ucode ever appears.
"""

from contextlib import ExitStack

import numpy as np

import concourse.bacc as bacc
import concourse.bass as bass
import concourse.tile as tile
from concourse import mybir
from concourse.bass_utils import run_bass_kernel_spmd

B, S, D = 16, 4096, 512
NCORES = 8
BPC = B // NCORES  # batches per core = 2
P = 128            # SBUF partitions
CPB = S // P       # chunks per batch = 32
import os

# Tapered x-DMA groups per batch (chunks of 256KB each). Lane reuse beyond
# the 8 DMAHW sem lanes is safe (verified on HW); the taper minimizes the
# PE-matmul chain left after the final byte lands (see module docstring).
GROUPS = [int(g) for g in os.environ.get("K_GROUPS", "18,8,4,1,1").split(",")]
assert sum(GROUPS) == CPB
# x-loads issue on the ACT HWDGE ring: reproducibly ~0.7us faster than sync
# (58.8/58.8 vs 59.4/59.6 us) — tok/out stay on SP, splitting sequencer work.
X_ENGINE = os.environ.get("K_XENG", "act")  # sync | act | gpsimd
# Tiles are tagged by group size; BPC bufs per tag gives every x DMA a private
# slot (no reuse -> no extra WAW waits). Total SBUF: 2*(16+15+1)*2KB = 128KB/p.
XBUFS = BPC
# PE HAM warming: "pre,per-group" dummy fp32 N=1 matmuls on `ones`. PE idles
# ~3.4us between DMA-paced groups, exactly the HAM re-gate window, so real
# matmuls run at the cold 1.2 GHz clock; dummies keep the PE busy -> 2.4 GHz.
WARM0, WARMG = (int(v) for v in os.environ.get("K_WARM", "0,0").split(","))

_NC = None


RAW = os.environ.get("K_RAW", "1") == "1"


def _build_nc():
    # Bacc (not plain Bass): its compile() runs generate_event_semaphores,
    # which splits multi-wait instructions onto InstEventSemaphore — TRN2
    # instructions can carry at most one sem wait.
    nc = bacc.Bacc(trn_type="TRN2")
    x = nc.dram_tensor("x", [BPC, S, D], mybir.dt.float32, kind="ExternalInput")
    tokens = nc.dram_tensor("tokens", [BPC, S], mybir.dt.int32, kind="ExternalInput")
    out = nc.dram_tensor("out", [BPC, D], mybir.dt.float32, kind="ExternalOutput")

    # s = p*CPB + c : per-partition contiguous rows
    xa = x[:].rearrange("b (p c) d -> b p c d", p=P)   # [BPC, 128, 32, 512]
    ta = tokens[:].rearrange("b (p c) -> p b c", p=P)  # [128, BPC, 32]
    oa = out[:].rearrange("b d -> (b d)")              # [BPC*512]

    if RAW:
        _raw_body(nc, xa, ta, oa)
    else:
        with TileKernel(nc) as tk:
            tk.body(xa, ta, oa)
    nc.compile()
    return nc


def _raw_body(nc, xa, ta, oa):
    """Hand-scheduled variant: no TileContext, manual semaphores.

    Why: Tile's entry barrier + event-sem config delays the first x-DMA
    trigger by ~0.8-1.4us after the measured window opens, and its exit adds
    a sem ladder. Raw emission lets the ACT engine's FIRST instruction be
    the first x-DMA trigger. SBUF is big enough to hold both batches'
    x (64 chunks x 2KB = 128KB/partition), so there is no buffer reuse and
    the only cross-engine sync needed is:

      SP:  tok DMA -> tsem          ACT: 10 x-DMA triggers -> xsem (16/DMA)
      DVE: [tsem] valid, rowsum -> vsem; [csem] recips -> rsem
      PE:  [vsem] cnt matmul -> csem; per group g: [xsem >= 16*g] matmuls;
           per-batch last matmul -> nsem
      ACT: [rsem, nsem>=b+1] orow_b = num_b * recip_b (reads PSUM) -> osem
      SP:  [osem >= b+1] store orow_b

    Cumulative xsem waits are sound on one HWDGE ring: each of the 16 SDMA
    engines emits 1 inc per DMA and processes its queue in ring order, so
    xsem >= 16*k implies every engine finished its share of the first k
    DMAs, i.e. DMAs 1..k fully landed.
    """
    with ExitStack() as es:
        sb = lambda name, shape, dt: es.enter_context(nc.sbuf_tensor(name, shape, dt))
        ps = lambda name, shape, dt: es.enter_context(nc.psum_tensor(name, shape, dt))
        sem = lambda name: es.enter_context(nc.semaphore(name))

        xsb = sb("xsb", [P, BPC * CPB, D], mybir.dt.float32r)  # both batches
        tok = sb("tok", [P, BPC, CPB], mybir.dt.int32)
        valid = sb("valid", [P, BPC, CPB], mybir.dt.float32r)
        rowsum = sb("rowsum", [P, BPC], mybir.dt.float32)
        recips = sb("recips", [1, BPC], mybir.dt.float32)
        orow = sb("orow", [1, BPC * D], mybir.dt.float32)
        ones = sb("ones", [P, 1], mybir.dt.float32)
        cnt = ps("cnt", [1, BPC], mybir.dt.float32)
        nums = [ps(f"num{b}", [1, D], mybir.dt.float32) for b in range(BPC)]

        nx = BPC * len(GROUPS)
        xsems = [sem(f"xsem{i}") for i in range(nx)]
        tsem = sem("tsem")
        vsem = sem("vsem")
        csem = sem("csem")
        rsem = sem("rsem")
        nsem = sem("nsem")
        osem = sem("osem")

        # --- x stream, queued up-front. Group 0 rides the SP ring (ahead of
        # tok) so its descriptor-gen runs in parallel with ACT's first
        # group's descgen -- the stream's first byte lands ~0.35us earlier.
        # The two rings share the 16 SDMA engines, so aggregate rate and the
        # last-byte time are unchanged; per-DMA sems make completion order
        # across rings irrelevant. tok still lands ~9us, well before the
        # DVE consumes it. --------------------------------------------------
        di = 0
        for b in range(BPC):
            c0 = 0
            for gi, grp in enumerate(GROUPS):
                eng = nc.sync if (b == 0 and gi == 0) else nc.scalar
                eng.dma_start(
                    out=xsb[:, b * CPB + c0:b * CPB + c0 + grp, :],
                    in_=xa[b, :, c0:c0 + grp, :].bitcast(mybir.dt.float32r),
                ).then_inc(xsems[di], 16)
                di += 1
                c0 += grp

        # --- SP: token load (behind x group 0) -----------------------------
        nc.sync.dma_start(out=tok[:], in_=ta).then_inc(tsem, 16)

        # --- DVE: mask, row sums, reciprocals ------------------------------
        # dsem: the race model doesn't credit same-engine program order, so
        # DVE's own write->read chain gets an explicit handshake.
        dsem = sem("dsem")
        nc.vector.memset(ones[:], 1.0).then_inc(dsem, 1)
        nc.vector.wait_ge(tsem, 16)
        nc.vector.tensor_scalar(
            out=valid[:], in0=tok[:], scalar1=0, scalar2=None,
            op0=mybir.AluOpType.not_equal,
        ).then_inc(dsem, 1)
        nc.vector.wait_ge(dsem, 2)
        nc.vector.reduce_sum(
            out=rowsum[:], in_=valid[:].bitcast(mybir.dt.float32),
            axis=mybir.AxisListType.X,
        ).then_inc(vsem, 1)
        nc.vector.wait_ge(csem, 1)
        nc.vector.reciprocal(recips[:], cnt[:]).then_inc(rsem, 1)

        # --- PE: counts, then the masked-sum groups ------------------------
        nc.tensor.wait_ge(vsem, 1)
        nc.tensor.matmul(cnt[:], ones[:], rowsum[:], start=True, stop=True
                         ).then_inc(csem, 1)
        dma_idx = 0
        for b in range(BPC):
            c0 = 0
            for grp in GROUPS:
                nc.tensor.wait_ge(xsems[dma_idx], 16)
                dma_idx += 1
                for k in range(grp):
                    c = c0 + k
                    mm = nc.tensor.matmul(
                        nums[b][:], valid[:, b, c:c + 1],
                        xsb[:, b * CPB + c, :],
                        start=(c == 0), stop=(c == CPB - 1),
                    )
                    if c == CPB - 1:
                        mm.then_inc(nsem, 1)
                c0 += grp

        # --- ACT: divide straight out of PSUM ------------------------------
        # (A split-divide across ACT+DVE and dropping the final fsem waits
        # were both tried; HW runs failed with redacted INTERNAL errors, so
        # this block stays exactly as the proven 53329ns version.)
        nc.scalar.wait_ge(rsem, 1)
        for b in range(BPC):
            nc.scalar.wait_ge(nsem, b + 1)
            nc.scalar.activation(
                orow[:, b * D:(b + 1) * D], nums[b][:],
                mybir.ActivationFunctionType.Copy, scale=recips[:, b:b + 1],
            ).then_inc(osem, 1)

        # --- SP: per-batch stores ------------------------------------------
        # The final fsem waits are REQUIRED: ending the program with these
        # DMAs in flight crashes the runtime at ring teardown (tested).
        fsems = [sem(f"fsem{b}") for b in range(BPC)]
        for b in range(BPC):
            nc.sync.wait_ge(osem, b + 1)
            nc.sync.dma_start(
                out=oa[b * D:(b + 1) * D], in_=orow[:, b * D:(b + 1) * D]
            ).then_inc(fsems[b], 16)
        for b in range(BPC):
            nc.sync.wait_ge(fsems[b], 16)


class TileKernel:
    def __init__(self, nc):
        self.nc = nc
        self.ctx = ExitStack()
        self.tc = None

    def __enter__(self):
        self.tc = self.ctx.enter_context(tile.TileContext(self.nc))
        return self

    def __exit__(self, *exc):
        return self.ctx.__exit__(*exc)

    def body(self, xa, ta, oa, xt0=None, xsem=None):
        nc = self.nc
        tc = self.tc
        ctx = self.ctx

        xpool = ctx.enter_context(tc.tile_pool(name="xpool", bufs=XBUFS))
        vpool = ctx.enter_context(tc.tile_pool(name="vpool", bufs=2))
        spool = ctx.enter_context(tc.tile_pool(name="spool", bufs=2))
        singles = ctx.enter_context(tc.tile_pool(name="singles", bufs=1))
        psum = ctx.enter_context(tc.tile_pool(name="psum", bufs=2, space="PSUM"))

        ones = singles.tile([P, 1], mybir.dt.float32)
        nc.vector.memset(ones, 1.0)
        xeng = {"sync": nc.sync, "act": nc.scalar, "gpsimd": nc.gpsimd}[X_ENGINE]

        warm = (
            psum.tile([1, 1], mybir.dt.float32, tag="warm", name="warm")
            if WARM0 or WARMG else None
        )

        def warm_pe(n):
            for _ in range(n):
                nc.tensor.matmul(warm, ones, ones, start=True, stop=True)

        warm_pe(WARM0)  # cover the initial DMA fill phase

        # --- mask + counts for both batches (one tok DMA) --------------------
        tok = vpool.tile([P, BPC, CPB], mybir.dt.int32)
        nc.sync.dma_start(out=tok, in_=ta)
        # valid is declared float32r so the fp32r matmul's verifier sees a
        # rounded producer; its values (0.0/1.0) are exact in any precision.
        valid = vpool.tile([P, BPC, CPB], mybir.dt.float32r)
        nc.vector.tensor_scalar(
            out=valid, in0=tok, scalar1=0, scalar2=None,
            op0=mybir.AluOpType.not_equal,
        )
        rowsum = spool.tile([P, BPC], mybir.dt.float32)
        nc.vector.reduce_sum(
            out=rowsum, in_=valid.bitcast(mybir.dt.float32),
            axis=mybir.AxisListType.X,
        )

        # both batches' counts in one matmul, reciprocals in one DVE op
        cnt = psum.tile([1, BPC], mybir.dt.float32)
        nc.tensor.matmul(cnt, ones, rowsum, start=True, stop=True)
        recips = spool.tile([1, BPC], mybir.dt.float32)
        nc.vector.reciprocal(recips, cnt)

        for b in range(BPC):
            recip = recips[:, b:b + 1]

            # --- masked sum ---------------------------------------------------
            num = psum.tile([1, D], mybir.dt.float32)
            c0 = 0
            for gi, grp in enumerate(GROUPS):
                if b == 0 and gi == 0 and xt0 is not None:
                    # first group was DMA'd by the hoisted pre-Tile transfer
                    nc.tensor.wait_ge(xsem, 16)
                    xt = xt0
                else:
                    # float32r: single-pass fp32 matmul (4x faster than fp32's
                    # two half-rate passes). Same 4-byte layout as fp32 so the
                    # DMA is a pure bit copy; the PE truncates low mantissa
                    # bits, mask weights are exact 0/1, PSUM accums in fp32.
                    xt = xpool.tile([P, grp, D], mybir.dt.float32r, tag=f"xt{grp}")
                    xeng.dma_start(out=xt, in_=xa[b, :, c0:c0 + grp, :].bitcast(mybir.dt.float32r))
                for k in range(grp):
                    c = c0 + k
                    nc.tensor.matmul(
                        num, valid[:, b, c:c + 1], xt[:, k, :],
                        start=(c == 0), stop=(c == CPB - 1),
                    )
                c0 += grp
                # keep PE busy through the wait for the next group's DMA,
                # except on the home stretch where dummies would delay the tail
                if not (b == BPC - 1 and c0 > CPB - GROUPS[-1] - GROUPS[-2]):
                    warm_pe(WARMG)

            # --- divide + store this batch's row as soon as it's ready --------
            # ACT reads PSUM directly: out = num * recip in one op, then store.
            orow = spool.tile([1, D], mybir.dt.float32, tag=f"orow{b}")
            nc.scalar.activation(
                orow, num, mybir.ActivationFunctionType.Copy, scale=recip
            )
            nc.sync.dma_start(out=oa[b * D:(b + 1) * D], in_=orow)


def _get_nc():
    global _NC
    if _NC is None:
        _NC = _build_nc()
    return _NC


def _shard(x, tokens):
    x = np.ascontiguousarray(np.asarray(x, dtype=np.float32))
    tokens = np.ascontiguousarray(np.asarray(tokens, dtype=np.int32))
    return [
        {
            "x": x[c * BPC:(c + 1) * BPC],
            "tokens": tokens[c * BPC:(c + 1) * BPC],
        }
        for c in range(NCORES)
    ]


def kernel(x, tokens):
    res = run_bass_kernel_spmd(_get_nc(), _shard(x, tokens), core_ids=list(range(NCORES)))
    return np.concatenate([r["out"] for r in res.results], axis=0)


def _install_ntff_shim():
    """The agent image's antenv lacks axon_hooks, so bass_utils' trace path
    can't find the NTFF hook. Recreate the tiny get/set module and register
    trn_boot's ctypes-based hook against the injected libaxon_pjrt.so."""
    import sys
    import types

    if "antenv.axon_hooks" in sys.modules:
        return
    mod = types.ModuleType("antenv.axon_hooks")
    state = {"hook": None}
    mod.set_axon_ntff_profile_hook = lambda h: state.__setitem__("hook", h)
    mod.get_axon_ntff_profile_hook = lambda: state["hook"]
    sys.modules["antenv.axon_hooks"] = mod
    try:
        from trn_agent_boot.trn_boot import _ntff_profile_via_ctypes

        mod.set_axon_ntff_profile_hook(
            _ntff_profile_via_ctypes("/opt/axon/libaxon_pjrt.so")
        )
    except Exception:
        pass


def kernel_profiled(x, tokens):
    """Same as kernel() but with NTFF tracing; returns (out, BassKernelResults)."""
    _install_ntff_shim()
    res = run_bass_kernel_spmd(
        _get_nc(), _shard(x, tokens), core_ids=list(range(NCORES)), trace=True
    )
    out = np.concatenate([r["out"] for r in res.results], axis=0)
    return out, res

